# revision 31
# baseline (speedup 1.0000x reference)
"""Trainium2 Bass kernel for nn_Attention_31705448579931.

Multi-head attention (b=16, L=784, dim=384, H=8, qk=32, v=128) with a
bicubic-resampled relative-position bias:

    out = proj( softmax(q k^T/sqrt(d) + M ab M^T) v )

Sharding: data-parallel over batch — each of the 8 NeuronCores handles 2
batches and all 8 heads. The (batch-independent) bias interpolation is
replicated per core, computed head-at-a-time and fused into softmax via
exp(S+B) = exp(S) * exp(B).

Device layout highlights:
  - attention is computed k-major: S^T tiles (kpos on partitions, q on free)
    so exp(S^T) is directly the lhsT-side operand for the P@V matmul.
  - softmax denominators via an ones-vector matmul on the TensorEngine,
    reciprocal on VectorE, partition-broadcast on GpSimd, and a column-scale
    fused into the PSUM->SBUF eviction of the attention output.
  - the bias interp phase 2 exploits the 4-tap bicubic structure of the
    interpolation matrix (contraction 625 -> ~96 rows per output tile).

All matmuls run in bf16 (inputs pre-cast on host) except the two bias-interp
phases which run float32r over f32 data.
"""

import numpy as np
import ml_dtypes

import concourse.bass as bass
import concourse.mybir as mybir
import concourse.tile as tile
from concourse import bacc
from concourse.bass_utils import run_bass_kernel_spmd

N_CORES = 8
B = 16          # global batch
BC = B // N_CORES  # batches per core
L = 784
DIM = 384
H = 8
QK = 32
VD = 128
RES = 25
N = RES * RES   # 625
A_CUBIC = -0.75
SCALE = QK ** -0.5

LT = 7          # l tiles of 112
LTS = 112
NT = 5          # 625 tiles of 125
NTS = 125
F32 = mybir.dt.float32
F32R = mybir.dt.float32r
BF16 = mybir.dt.bfloat16
BF16_NP = ml_dtypes.bfloat16

NSPLITS = [(0, 512), (512, L)]  # free-dim chunks for 784-wide matmul outputs


def _cubic_weight(x):
    ax = np.abs(x)
    a = A_CUBIC
    w1 = ((a + 2.0) * ax - (a + 3.0)) * ax * ax + 1.0
    w2 = a * (((ax - 5.0) * ax + 8.0) * ax - 4.0)
    return np.where(ax <= 1.0, w1, np.where(ax < 2.0, w2, 0.0)).astype(np.float32)


def interp_matrix(Lo, Li):
    """Dense 1-D bicubic resampling matrix (Lo, Li), matches reference."""
    scale = Li / Lo
    src = (np.arange(Lo, dtype=np.float32) + 0.5) * scale - 0.5
    f = np.floor(src)
    t = (src - f).astype(np.float32)
    ws = np.stack(
        [_cubic_weight(t + 1.0), _cubic_weight(t), _cubic_weight(1.0 - t),
         _cubic_weight(2.0 - t)], axis=1)
    idx = f.astype(np.int32)[:, None] + np.arange(-1, 3, dtype=np.int32)[None, :]
    idx = np.clip(idx, 0, Li - 1)
    M = np.zeros((Lo, Li), dtype=np.float32)
    np.add.at(M, (np.arange(Lo)[:, None], idx), ws)
    return M


def _tap_windows():
    """Per l-tile row window [lo, hi) of MT rows feeding that tile (4-tap)."""
    scale = N / L
    src = (np.arange(L, dtype=np.float64) + 0.5) * scale - 0.5
    f = np.floor(src).astype(np.int64)
    lo_tap = np.clip(f - 1, 0, N - 1)
    hi_tap = np.clip(f + 2, 0, N - 1)
    wins = []
    for pt in range(LT):
        sl = slice(pt * LTS, (pt + 1) * LTS)
        wins.append((int(lo_tap[sl].min()), int(hi_tap[sl].max()) + 1))
    return wins


_BUILD_CACHE = {}


def build():
    if "nc" in _BUILD_CACHE:
        return _BUILD_CACHE["nc"]

    nc = bacc.Bacc("TRN2", target_bir_lowering=False, debug=False,
                   num_devices=N_CORES)

    xT_e = nc.dram_tensor("xT", [BC, DIM, L], BF16, kind="ExternalInput")
    wqkvT_e = nc.dram_tensor("wqkvT", [DIM, 1536], BF16, kind="ExternalInput")
    wprojT_e = nc.dram_tensor("wprojT", [H * VD, DIM], BF16, kind="ExternalInput")
    bproj_e = nc.dram_tensor("bproj", [1, DIM], F32, kind="ExternalInput")
    mt_e = nc.dram_tensor("mt", [N, L], BF16, kind="ExternalInput")
    ab_e = nc.dram_tensor("ab", [N, N], BF16, kind="ExternalInput")
    ebloc_e = nc.dram_tensor("ebloc", [L, L], BF16)
    eball_e = nc.dram_tensor("eball", [H * L, L], BF16, addr_space="Shared")
    out_e = nc.dram_tensor("out", [BC, L, DIM], F32, kind="ExternalOutput")

    wins = _tap_windows()

    with tile.TileContext(nc) as tc:
        with (
            tc.tile_pool(name="const", bufs=1) as constp,
            tc.tile_pool(name="wq", bufs=1) as wqp,
            tc.tile_pool(name="x", bufs=1) as xp,
            tc.tile_pool(name="v", bufs=1) as vp,
            tc.tile_pool(name="qk", bufs=1) as qkp,
            tc.tile_pool(name="bias", bufs=1) as biasp,
            tc.tile_pool(name="attn", bufs=2) as attnp,
            tc.tile_pool(name="flow", bufs=8) as flowp,
            tc.tile_pool(name="mis", bufs=1) as misp,
            tc.tile_pool(name="ot", bufs=1) as otp,
            tc.tile_pool(name="ps", bufs=4, space="PSUM") as psp,
        ):
            # ---- constants -------------------------------------------------
            bproj_row = constp.tile([1, DIM], F32, tag="bprow")
            nc.sync.dma_start(bproj_row[:], bproj_e[:, :])
            bpb = constp.tile([128, DIM], F32, tag="bpb")
            nc.gpsimd.partition_broadcast(bpb[:], bproj_row[:])

            ones_l = constp.tile([LTS, VD], BF16, tag="ones")
            nc.any.memset(ones_l[:], 1.0)

            # weights
            wq_sb = []
            for kc in range(3):
                t = wqp.tile([128, 1536], BF16, tag=f"wq{kc}")
                nc.sync.dma_start(t[:], wqkvT_e[kc * 128:(kc + 1) * 128, :])
                wq_sb.append(t)
            wproj_sb = []
            for h in range(H):
                t = wqp.tile([128, DIM], BF16, tag=f"wp{h}")
                nc.sync.dma_start(t[:], wprojT_e[h * VD:(h + 1) * VD, :])
                wproj_sb.append(t)


            # x^T
            xT_sb = [[None] * 3 for _ in range(BC)]
            for b in range(BC):
                for kc in range(3):
                    t = xp.tile([128, L], BF16, tag=f"x{b}{kc}")
                    nc.sync.dma_start(t[:], xT_e[b, kc * 128:(kc + 1) * 128, :])
                    xT_sb[b][kc] = t

            # ---- own-head bias: expB^T = exp(M ab^T M^T), then AllGather --
            # scoped pool: all bias-phase operands are freed once the
            # collective is issued.
            with tc.tile_pool(name="bias1", bufs=1) as b1p, \
                    tc.tile_pool(name="uwp", bufs=2) as uwp:
                mt_sb = []
                for ic in range(NT):
                    t = b1p.tile([NTS, L], BF16, tag=f"mt{ic}")
                    nc.sync.dma_start(t[:], mt_e[ic * NTS:(ic + 1) * NTS, :])
                    mt_sb.append(t)
                mtwin_sb = []
                for pt in range(LT):
                    lo, hi = wins[pt]
                    t = b1p.tile([hi - lo, LTS], BF16, tag=f"mtw{pt}")
                    nc.sync.dma_start(t[:], mt_e[lo:hi, pt * LTS:(pt + 1) * LTS])
                    mtwin_sb.append(t)
                ab_sb = []
                for ic in range(NT):
                    t = b1p.tile([NTS, N], BF16, tag=f"ab{ic}")
                    nc.gpsimd.dma_start(t[:], ab_e[ic * NTS:(ic + 1) * NTS, :])
                    ab_sb.append(t)
                for pt in range(LT):
                    lo, hi = wins[pt]
                    pu = psp.tile([hi - lo, L], F32, tag="ps")
                    for (ns, ne) in NSPLITS:
                        for ic in range(NT):
                            nc.tensor.matmul(
                                pu[:, ns:ne],
                                lhsT=ab_sb[ic][:, lo:hi],
                                rhs=mt_sb[ic][:, ns:ne],
                                start=(ic == 0), stop=(ic == NT - 1),
                            )
                    uw = uwp.tile([hi - lo, L], BF16, tag="uw")
                    nc.vector.tensor_copy(uw[:], pu[:])
                    pb = psp.tile([LTS, L], F32, tag="ps")
                    for (ns, ne) in NSPLITS:
                        nc.tensor.matmul(
                            pb[:, ns:ne],
                            lhsT=mtwin_sb[pt][:],
                            rhs=uw[:, ns:ne],
                            start=True, stop=True,
                        )
                    ebo = b1p.tile([LTS, L], BF16, tag="ebo")
                    nc.scalar.activation(ebo[:], pb[:],
                                         mybir.ActivationFunctionType.Exp)
                    nc.gpsimd.dma_start(ebloc_e[pt * LTS:(pt + 1) * LTS, :], ebo[:])
                nc.gpsimd.collective_compute(
                    "AllGather",
                    mybir.AluOpType.bypass,
                    replica_groups=[list(range(N_CORES))],
                    ins=[ebloc_e.ap().opt()],
                    outs=[eball_e.ap().opt()],
                )

            # ---- V projection (l-major: kpos on partitions) ---------------
            v_sb = [[None] * LT for _ in range(BC)]
            for b in range(BC):
                for lt in range(LT):
                    pv = psp.tile([LTS, 1024], F32, tag="ps")
                    for half in range(2):
                        for kc in range(3):
                            nc.tensor.matmul(
                                pv[:, half * 512:(half + 1) * 512],
                                lhsT=xT_sb[b][kc][:, lt * LTS:(lt + 1) * LTS],
                                rhs=wq_sb[kc][:, 512 + half * 512:1024 + half * 512],
                                start=(kc == 0), stop=(kc == 2),
                            )
                    vt = vp.tile([LTS, 1024], BF16, tag=f"v{b}{lt}")
                    nc.scalar.copy(vt[:], pv[:])
                    v_sb[b][lt] = vt

            # ---- q/k projections for all heads (hoisted so this PE work
            # overlaps the AllGather) ---------------------------------------
            qk_all = {}
            for mt_i in range(H // 2):
                for b in range(BC):
                    pqk = psp.tile([128, L], F32, tag="ps")
                    for (ns, ne) in NSPLITS:
                        for kc in range(3):
                            nc.tensor.matmul(
                                pqk[:, ns:ne],
                                lhsT=wq_sb[kc][:, mt_i * 128:(mt_i + 1) * 128],
                                rhs=xT_sb[b][kc][:, ns:ne],
                                start=(kc == 0), stop=(kc == 2),
                            )
                    for hh in range(2):
                        h_ = 2 * mt_i + hh
                        qt = qkp.tile([2 * QK, L], BF16, tag=f"q{b}{h_}")
                        kt_ = qkp.tile([2 * QK, L], BF16, tag=f"k{b}{h_}")
                        nc.vector.tensor_copy(qt[0:QK, :], pqk[hh * 64:hh * 64 + 32, :])
                        nc.vector.tensor_copy(kt_[0:QK, :], pqk[hh * 64 + 32:hh * 64 + 64, :])
                        # duplicate rows into partitions 32:64 for 2-way
                        # row-group packing of the S^T matmuls
                        nc.sync.dma_start(qt[QK:2 * QK, :], qt[0:QK, :])
                        nc.sync.dma_start(kt_[QK:2 * QK, :], kt_[0:QK, :])
                        qk_all[(b, h_)] = (qt, kt_)

            # ---- per-head loop --------------------------------------------
            ot_sb = [[None] * H for _ in range(BC)]
            _pend = []

            def _flush_pv(item):
                fh, fb, ptiles = item
                ps_o = psp.tile([VD, L], F32, tag="ps")
                ps_one = psp.tile([VD, L], F32, tag="ps")
                # ones chains first (M=128: same N-bound cost as M=1, but the
                # output replicates the column sums across all partitions --
                # no partition broadcast needed and the reciprocal runs on
                # all 128 lanes).
                for (ns, ne) in NSPLITS:
                    for kt in range(LT):
                        nc.tensor.matmul(
                            ps_one[:, ns:ne],
                            lhsT=ones_l[:],
                            rhs=ptiles[kt][:, ns:ne],
                            start=(kt == 0), stop=(kt == LT - 1),
                        )
                rdb = misp.tile([VD, L], F32, tag="rdb")
                nc.vector.reciprocal_approx_fast(rdb[:], ps_one[:])
                for (ns, ne) in NSPLITS:
                    for kt in range(LT):
                        nc.tensor.matmul(
                            ps_o[:, ns:ne],
                            lhsT=v_sb[fb][kt][:, fh * VD:(fh + 1) * VD],
                            rhs=ptiles[kt][:, ns:ne],
                            start=(kt == 0), stop=(kt == LT - 1),
                        )
                ot = otp.tile([VD, L], BF16, tag=f"ot{fb}{fh}")
                nc.vector.tensor_mul(ot[:], ps_o[:], rdb[:])
                ot_sb[fb][fh] = ot
            for h in range(H):
                # --- bias for head h: load gathered expB^T tiles -----------
                expb_sb = []
                for pt in range(LT):
                    eb = biasp.tile([LTS, L], BF16, tag=f"eb{pt}")
                    nc.gpsimd.dma_start(
                        eb[:], eball_e[h * L + pt * LTS:h * L + (pt + 1) * LTS, :])
                    expb_sb.append(eb)

                # --- attention for (h, b): software-pipelined ---------------
                # Emit instance i's S^T/exp/mul, then flush instance i-1's
                # PV/ones chains: by the time the PE reaches a chain, every
                # pT operand is long ready, so the chain streams gap-free.
                for b in range(BC):
                    qt, kt_ = qk_all[(b, h)]
                    pt_tiles = []
                    for w0 in range(0, LT, 2):
                        wave = [w0] if w0 + 1 >= LT else [w0, w0 + 1]
                        pss = []
                        for wi, kt in enumerate(wave):
                            ps_s = psp.tile([LTS, L], F32, tag="ps")
                            pss.append(ps_s)
                        # row-group packed: wave member wi uses partitions
                        # [32*wi, 32*wi+32) of the duplicated q/k tiles, so
                        # the two K=32 matmuls run concurrently on the PE.
                        for (ns, ne) in NSPLITS:
                            for wi, kt in enumerate(wave):
                                o0 = wi * QK
                                nc.tensor.matmul(
                                    pss[wi][:, ns:ne],
                                    lhsT=kt_[o0:o0 + QK, kt * LTS:(kt + 1) * LTS],
                                    rhs=qt[o0:o0 + QK, ns:ne],
                                    start=True, stop=True,
                                )
                        for wi, kt in enumerate(wave):
                            es = flowp.tile([LTS, L], BF16, tag="expS")
                            nc.scalar.activation(es[:], pss[wi][:],
                                                 mybir.ActivationFunctionType.Exp)
                            ptile = attnp.tile([LTS, L], BF16, tag=f"pT{kt}")
                            nc.vector.tensor_mul(ptile[:], es[:], expb_sb[kt][:])
                            pt_tiles.append(ptile)
                    _pend.append((h, b, pt_tiles))
                    if len(_pend) > 1:
                        _flush_pv(_pend.pop(0))

            while _pend:
                _flush_pv(_pend.pop(0))

            # ---- output projection ----------------------------------------
            for b in range(BC):
                for lt in range(LT):
                    py = psp.tile([LTS, DIM], F32, tag="ps")
                    for h in range(H):
                        nc.tensor.matmul(
                            py[:],
                            lhsT=ot_sb[b][h][:, lt * LTS:(lt + 1) * LTS],
                            rhs=wproj_sb[h][:],
                            start=(h == 0), stop=(h == H - 1),
                        )
                    ysb = misp.tile([LTS, DIM], F32, tag="y")
                    nc.vector.tensor_add(ysb[:], py[:], bpb[:LTS, :])
                    nc.sync.dma_start(
                        out_e[b, lt * LTS:(lt + 1) * LTS, :], ysb[:])

    nc.compile()
    _BUILD_CACHE["nc"] = nc
    return nc


def _prep_in_maps(inputs):
    x = np.asarray(inputs["x"], dtype=np.float32)
    Wqkv = np.asarray(inputs["Wqkv"], dtype=np.float32)
    Wproj = np.asarray(inputs["Wproj"], dtype=np.float32)
    bproj = np.asarray(inputs["bproj"], dtype=np.float32)
    ab_table = np.asarray(inputs["ab_table"], dtype=np.float32)
    bias_idxs = np.asarray(inputs["bias_idxs"])

    # reorder qkv weight rows: [q0 k0 q1 k1 ... q7 k7 | v0 v1 ... v7]
    w3 = Wqkv.reshape(H, 2 * QK + VD, DIM)
    order = []
    for h in range(H):
        pass
    qk_rows = np.concatenate(
        [np.concatenate([w3[h, :QK] * SCALE, w3[h, QK:2 * QK]], axis=0)
         for h in range(H)],
        axis=0)                     # (512, 384)
    v_rows = np.concatenate([w3[h, 2 * QK:] for h in range(H)], axis=0)  # (1024,384)
    wqkvT = np.ascontiguousarray(
        np.concatenate([qk_rows, v_rows], axis=0).T).astype(BF16_NP)  # (384,1536)

    wprojT = np.ascontiguousarray(Wproj.T).astype(BF16_NP)  # (1024, 384)
    mt = np.ascontiguousarray(interp_matrix(L, N).T).astype(BF16_NP)  # (625, 784)
    ab_all = np.ascontiguousarray(ab_table[:, bias_idxs]).astype(BF16_NP)
    bproj2 = np.ascontiguousarray(bproj.reshape(1, DIM))

    in_maps = []
    for c in range(N_CORES):
        xT = np.ascontiguousarray(
            x[c * BC:(c + 1) * BC].transpose(0, 2, 1)).astype(BF16_NP)
        in_maps.append({
            "xT": xT,
            "wqkvT": wqkvT,
            "wprojT": wprojT,
            "bproj": bproj2,
            "mt": mt,
            "ab": ab_all[c],
        })
    return in_maps


def _run(inputs, trace=False, **kw):
    nc = build()
    in_maps = _prep_in_maps(inputs)
    res = run_bass_kernel_spmd(nc, in_maps, core_ids=list(range(N_CORES)),
                               trace=trace, **kw)
    out = np.concatenate([res.results[c]["out"] for c in range(N_CORES)], axis=0)
    return out, res


def kernel(**inputs) -> np.ndarray:
    out, _ = _run(inputs, trace=False)
    return out


# revision 32
# speedup vs baseline: 1.0066x; 1.0066x over previous
"""Trainium2 Bass kernel for nn_Attention_31705448579931.

Multi-head attention (b=16, L=784, dim=384, H=8, qk=32, v=128) with a
bicubic-resampled relative-position bias:

    out = proj( softmax(q k^T/sqrt(d) + M ab M^T) v )

Sharding: data-parallel over batch — each of the 8 NeuronCores handles 2
batches and all 8 heads. The (batch-independent) bias interpolation is
replicated per core, computed head-at-a-time and fused into softmax via
exp(S+B) = exp(S) * exp(B).

Device layout highlights:
  - attention is computed k-major: S^T tiles (kpos on partitions, q on free)
    so exp(S^T) is directly the lhsT-side operand for the P@V matmul.
  - softmax denominators via an ones-vector matmul on the TensorEngine,
    reciprocal on VectorE, partition-broadcast on GpSimd, and a column-scale
    fused into the PSUM->SBUF eviction of the attention output.
  - the bias interp phase 2 exploits the 4-tap bicubic structure of the
    interpolation matrix (contraction 625 -> ~96 rows per output tile).

All matmuls run in bf16 (inputs pre-cast on host) except the two bias-interp
phases which run float32r over f32 data.
"""

import numpy as np
import ml_dtypes

import concourse.bass as bass
import concourse.mybir as mybir
import concourse.tile as tile
from concourse import bacc
from concourse.bass_utils import run_bass_kernel_spmd

N_CORES = 8
B = 16          # global batch
BC = B // N_CORES  # batches per core
L = 784
DIM = 384
H = 8
QK = 32
VD = 128
RES = 25
N = RES * RES   # 625
A_CUBIC = -0.75
SCALE = QK ** -0.5

LT = 7          # l tiles of 112
LTS = 112
NT = 5          # 625 tiles of 125
NTS = 125
F32 = mybir.dt.float32
F32R = mybir.dt.float32r
BF16 = mybir.dt.bfloat16
BF16_NP = ml_dtypes.bfloat16

NSPLITS = [(0, 512), (512, L)]  # free-dim chunks for 784-wide matmul outputs


def _cubic_weight(x):
    ax = np.abs(x)
    a = A_CUBIC
    w1 = ((a + 2.0) * ax - (a + 3.0)) * ax * ax + 1.0
    w2 = a * (((ax - 5.0) * ax + 8.0) * ax - 4.0)
    return np.where(ax <= 1.0, w1, np.where(ax < 2.0, w2, 0.0)).astype(np.float32)


def interp_matrix(Lo, Li):
    """Dense 1-D bicubic resampling matrix (Lo, Li), matches reference."""
    scale = Li / Lo
    src = (np.arange(Lo, dtype=np.float32) + 0.5) * scale - 0.5
    f = np.floor(src)
    t = (src - f).astype(np.float32)
    ws = np.stack(
        [_cubic_weight(t + 1.0), _cubic_weight(t), _cubic_weight(1.0 - t),
         _cubic_weight(2.0 - t)], axis=1)
    idx = f.astype(np.int32)[:, None] + np.arange(-1, 3, dtype=np.int32)[None, :]
    idx = np.clip(idx, 0, Li - 1)
    M = np.zeros((Lo, Li), dtype=np.float32)
    np.add.at(M, (np.arange(Lo)[:, None], idx), ws)
    return M


def _tap_windows():
    """Per l-tile row window [lo, hi) of MT rows feeding that tile (4-tap)."""
    scale = N / L
    src = (np.arange(L, dtype=np.float64) + 0.5) * scale - 0.5
    f = np.floor(src).astype(np.int64)
    lo_tap = np.clip(f - 1, 0, N - 1)
    hi_tap = np.clip(f + 2, 0, N - 1)
    wins = []
    for pt in range(LT):
        sl = slice(pt * LTS, (pt + 1) * LTS)
        wins.append((int(lo_tap[sl].min()), int(hi_tap[sl].max()) + 1))
    return wins


_BUILD_CACHE = {}


def build():
    if "nc" in _BUILD_CACHE:
        return _BUILD_CACHE["nc"]

    nc = bacc.Bacc("TRN2", target_bir_lowering=False, debug=False,
                   num_devices=N_CORES)

    xT_e = nc.dram_tensor("xT", [BC, DIM, L], BF16, kind="ExternalInput")
    wqkvT_e = nc.dram_tensor("wqkvT", [DIM, 1536], BF16, kind="ExternalInput")
    wprojT_e = nc.dram_tensor("wprojT", [H * VD, DIM], BF16, kind="ExternalInput")
    bproj_e = nc.dram_tensor("bproj", [1, DIM], F32, kind="ExternalInput")
    mt_e = nc.dram_tensor("mt", [N, L], BF16, kind="ExternalInput")
    ab_e = nc.dram_tensor("ab", [N, N], BF16, kind="ExternalInput")
    ebloc_e = nc.dram_tensor("ebloc", [L, L], BF16)
    eball_e = nc.dram_tensor("eball", [H * L, L], BF16, addr_space="Shared")
    out_e = nc.dram_tensor("out", [BC, L, DIM], F32, kind="ExternalOutput")

    wins = _tap_windows()

    with tile.TileContext(nc) as tc:
        with (
            tc.tile_pool(name="const", bufs=1) as constp,
            tc.tile_pool(name="wq", bufs=1) as wqp,
            tc.tile_pool(name="x", bufs=1) as xp,
            tc.tile_pool(name="v", bufs=1) as vp,
            tc.tile_pool(name="qk", bufs=1) as qkp,
            tc.tile_pool(name="bias", bufs=1) as biasp,
            tc.tile_pool(name="attn", bufs=2) as attnp,
            tc.tile_pool(name="flow", bufs=8) as flowp,
            tc.tile_pool(name="mis", bufs=1) as misp,
            tc.tile_pool(name="ot", bufs=1) as otp,
            tc.tile_pool(name="ps", bufs=4, space="PSUM") as psp,
        ):
            # ---- constants -------------------------------------------------
            bproj_row = constp.tile([1, DIM], F32, tag="bprow")
            nc.sync.dma_start(bproj_row[:], bproj_e[:, :])
            bpb = constp.tile([128, DIM], F32, tag="bpb")
            nc.gpsimd.partition_broadcast(bpb[:], bproj_row[:])

            ones_l = constp.tile([LTS, VD], BF16, tag="ones")
            nc.any.memset(ones_l[:], 1.0)

            # weights
            wq_sb = []
            for kc in range(3):
                t = wqp.tile([128, 1536], BF16, tag=f"wq{kc}")
                nc.sync.dma_start(t[:], wqkvT_e[kc * 128:(kc + 1) * 128, :])
                wq_sb.append(t)
            wproj_sb = []
            for h in range(H):
                t = wqp.tile([128, DIM], BF16, tag=f"wp{h}")
                nc.sync.dma_start(t[:], wprojT_e[h * VD:(h + 1) * VD, :])
                wproj_sb.append(t)


            # x^T
            xT_sb = [[None] * 3 for _ in range(BC)]
            for b in range(BC):
                for kc in range(3):
                    t = xp.tile([128, L], BF16, tag=f"x{b}{kc}")
                    nc.sync.dma_start(t[:], xT_e[b, kc * 128:(kc + 1) * 128, :])
                    xT_sb[b][kc] = t

            # ---- PE warmup: ~4us of dense matmuls releases the HAM clock
            # throttle (1.2 -> 2.4 GHz) before the latency-critical bias
            # phase that gates the AllGather.
            warm_ps = psp.tile([VD, 512], F32, tag="ps")
            for wi in range(20):
                nc.tensor.matmul(
                    warm_ps[:],
                    lhsT=ones_l[:],
                    rhs=wq_sb[0][:LTS, 0:512],
                    start=(wi == 0), stop=(wi == 19),
                )

            # ---- own-head bias: expB^T = exp(M ab^T M^T), then AllGather --
            # scoped pool: all bias-phase operands are freed once the
            # collective is issued.
            with tc.tile_pool(name="bias1", bufs=1) as b1p, \
                    tc.tile_pool(name="uwp", bufs=2) as uwp:
                mt_sb = []
                for ic in range(NT):
                    t = b1p.tile([NTS, L], BF16, tag=f"mt{ic}")
                    nc.sync.dma_start(t[:], mt_e[ic * NTS:(ic + 1) * NTS, :])
                    mt_sb.append(t)
                mtwin_sb = []
                for pt in range(LT):
                    lo, hi = wins[pt]
                    t = b1p.tile([hi - lo, LTS], BF16, tag=f"mtw{pt}")
                    nc.sync.dma_start(t[:], mt_e[lo:hi, pt * LTS:(pt + 1) * LTS])
                    mtwin_sb.append(t)
                ab_sb = []
                for ic in range(NT):
                    t = b1p.tile([NTS, N], BF16, tag=f"ab{ic}")
                    nc.gpsimd.dma_start(t[:], ab_e[ic * NTS:(ic + 1) * NTS, :])
                    ab_sb.append(t)
                for pt in range(LT):
                    lo, hi = wins[pt]
                    pu = psp.tile([hi - lo, L], F32, tag="ps")
                    for (ns, ne) in NSPLITS:
                        for ic in range(NT):
                            nc.tensor.matmul(
                                pu[:, ns:ne],
                                lhsT=ab_sb[ic][:, lo:hi],
                                rhs=mt_sb[ic][:, ns:ne],
                                start=(ic == 0), stop=(ic == NT - 1),
                            )
                    uw = uwp.tile([hi - lo, L], BF16, tag="uw")
                    nc.vector.tensor_copy(uw[:], pu[:])
                    pb = psp.tile([LTS, L], F32, tag="ps")
                    for (ns, ne) in NSPLITS:
                        nc.tensor.matmul(
                            pb[:, ns:ne],
                            lhsT=mtwin_sb[pt][:],
                            rhs=uw[:, ns:ne],
                            start=True, stop=True,
                        )
                    ebo = b1p.tile([LTS, L], BF16, tag="ebo")
                    nc.scalar.activation(ebo[:], pb[:],
                                         mybir.ActivationFunctionType.Exp)
                    nc.gpsimd.dma_start(ebloc_e[pt * LTS:(pt + 1) * LTS, :], ebo[:])
                nc.gpsimd.collective_compute(
                    "AllGather",
                    mybir.AluOpType.bypass,
                    replica_groups=[list(range(N_CORES))],
                    ins=[ebloc_e.ap().opt()],
                    outs=[eball_e.ap().opt()],
                )

            # ---- V projection (l-major: kpos on partitions) ---------------
            v_sb = [[None] * LT for _ in range(BC)]
            for b in range(BC):
                for lt in range(LT):
                    pv = psp.tile([LTS, 1024], F32, tag="ps")
                    for half in range(2):
                        for kc in range(3):
                            nc.tensor.matmul(
                                pv[:, half * 512:(half + 1) * 512],
                                lhsT=xT_sb[b][kc][:, lt * LTS:(lt + 1) * LTS],
                                rhs=wq_sb[kc][:, 512 + half * 512:1024 + half * 512],
                                start=(kc == 0), stop=(kc == 2),
                            )
                    vt = vp.tile([LTS, 1024], BF16, tag=f"v{b}{lt}")
                    nc.scalar.copy(vt[:], pv[:])
                    v_sb[b][lt] = vt

            # ---- q/k projections for all heads (hoisted so this PE work
            # overlaps the AllGather) ---------------------------------------
            qk_all = {}
            for mt_i in range(H // 2):
                for b in range(BC):
                    pqk = psp.tile([128, L], F32, tag="ps")
                    for (ns, ne) in NSPLITS:
                        for kc in range(3):
                            nc.tensor.matmul(
                                pqk[:, ns:ne],
                                lhsT=wq_sb[kc][:, mt_i * 128:(mt_i + 1) * 128],
                                rhs=xT_sb[b][kc][:, ns:ne],
                                start=(kc == 0), stop=(kc == 2),
                            )
                    for hh in range(2):
                        h_ = 2 * mt_i + hh
                        qt = qkp.tile([2 * QK, L], BF16, tag=f"q{b}{h_}")
                        kt_ = qkp.tile([2 * QK, L], BF16, tag=f"k{b}{h_}")
                        nc.vector.tensor_copy(qt[0:QK, :], pqk[hh * 64:hh * 64 + 32, :])
                        nc.vector.tensor_copy(kt_[0:QK, :], pqk[hh * 64 + 32:hh * 64 + 64, :])
                        # duplicate rows into partitions 32:64 for 2-way
                        # row-group packing of the S^T matmuls
                        nc.sync.dma_start(qt[QK:2 * QK, :], qt[0:QK, :])
                        nc.sync.dma_start(kt_[QK:2 * QK, :], kt_[0:QK, :])
                        qk_all[(b, h_)] = (qt, kt_)

            # ---- per-head loop --------------------------------------------
            ot_sb = [[None] * H for _ in range(BC)]
            _pend = []

            def _flush_pv(item):
                fh, fb, ptiles = item
                ps_o = psp.tile([VD, L], F32, tag="ps")
                ps_one = psp.tile([VD, L], F32, tag="ps")
                # ones chains first (M=128: same N-bound cost as M=1, but the
                # output replicates the column sums across all partitions --
                # no partition broadcast needed and the reciprocal runs on
                # all 128 lanes).
                for (ns, ne) in NSPLITS:
                    for kt in range(LT):
                        nc.tensor.matmul(
                            ps_one[:, ns:ne],
                            lhsT=ones_l[:],
                            rhs=ptiles[kt][:, ns:ne],
                            start=(kt == 0), stop=(kt == LT - 1),
                        )
                rdb = misp.tile([VD, L], F32, tag="rdb")
                nc.vector.reciprocal_approx_fast(rdb[:], ps_one[:])
                for (ns, ne) in NSPLITS:
                    for kt in range(LT):
                        nc.tensor.matmul(
                            ps_o[:, ns:ne],
                            lhsT=v_sb[fb][kt][:, fh * VD:(fh + 1) * VD],
                            rhs=ptiles[kt][:, ns:ne],
                            start=(kt == 0), stop=(kt == LT - 1),
                        )
                ot = otp.tile([VD, L], BF16, tag=f"ot{fb}{fh}")
                nc.vector.tensor_mul(ot[:], ps_o[:], rdb[:])
                ot_sb[fb][fh] = ot
            for h in range(H):
                # --- bias for head h: load gathered expB^T tiles -----------
                expb_sb = []
                for pt in range(LT):
                    eb = biasp.tile([LTS, L], BF16, tag=f"eb{pt}")
                    nc.gpsimd.dma_start(
                        eb[:], eball_e[h * L + pt * LTS:h * L + (pt + 1) * LTS, :])
                    expb_sb.append(eb)

                # --- attention for (h, b): software-pipelined ---------------
                # Emit instance i's S^T/exp/mul, then flush instance i-1's
                # PV/ones chains: by the time the PE reaches a chain, every
                # pT operand is long ready, so the chain streams gap-free.
                for b in range(BC):
                    qt, kt_ = qk_all[(b, h)]
                    pt_tiles = []
                    for w0 in range(0, LT, 2):
                        wave = [w0] if w0 + 1 >= LT else [w0, w0 + 1]
                        pss = []
                        for wi, kt in enumerate(wave):
                            ps_s = psp.tile([LTS, L], F32, tag="ps")
                            pss.append(ps_s)
                        # row-group packed: wave member wi uses partitions
                        # [32*wi, 32*wi+32) of the duplicated q/k tiles, so
                        # the two K=32 matmuls run concurrently on the PE.
                        for (ns, ne) in NSPLITS:
                            for wi, kt in enumerate(wave):
                                o0 = wi * QK
                                nc.tensor.matmul(
                                    pss[wi][:, ns:ne],
                                    lhsT=kt_[o0:o0 + QK, kt * LTS:(kt + 1) * LTS],
                                    rhs=qt[o0:o0 + QK, ns:ne],
                                    start=True, stop=True,
                                )
                        for wi, kt in enumerate(wave):
                            es = flowp.tile([LTS, L], BF16, tag="expS")
                            nc.scalar.activation(es[:], pss[wi][:],
                                                 mybir.ActivationFunctionType.Exp)
                            ptile = attnp.tile([LTS, L], BF16, tag=f"pT{kt}")
                            nc.vector.tensor_mul(ptile[:], es[:], expb_sb[kt][:])
                            pt_tiles.append(ptile)
                    _pend.append((h, b, pt_tiles))
                    if len(_pend) > 1:
                        _flush_pv(_pend.pop(0))

            while _pend:
                _flush_pv(_pend.pop(0))

            # ---- output projection ----------------------------------------
            for b in range(BC):
                for lt in range(LT):
                    py = psp.tile([LTS, DIM], F32, tag="ps")
                    for h in range(H):
                        nc.tensor.matmul(
                            py[:],
                            lhsT=ot_sb[b][h][:, lt * LTS:(lt + 1) * LTS],
                            rhs=wproj_sb[h][:],
                            start=(h == 0), stop=(h == H - 1),
                        )
                    ysb = misp.tile([LTS, DIM], F32, tag="y")
                    nc.vector.tensor_add(ysb[:], py[:], bpb[:LTS, :])
                    nc.sync.dma_start(
                        out_e[b, lt * LTS:(lt + 1) * LTS, :], ysb[:])

    nc.compile()
    _BUILD_CACHE["nc"] = nc
    return nc


def _prep_in_maps(inputs):
    x = np.asarray(inputs["x"], dtype=np.float32)
    Wqkv = np.asarray(inputs["Wqkv"], dtype=np.float32)
    Wproj = np.asarray(inputs["Wproj"], dtype=np.float32)
    bproj = np.asarray(inputs["bproj"], dtype=np.float32)
    ab_table = np.asarray(inputs["ab_table"], dtype=np.float32)
    bias_idxs = np.asarray(inputs["bias_idxs"])

    # reorder qkv weight rows: [q0 k0 q1 k1 ... q7 k7 | v0 v1 ... v7]
    w3 = Wqkv.reshape(H, 2 * QK + VD, DIM)
    order = []
    for h in range(H):
        pass
    qk_rows = np.concatenate(
        [np.concatenate([w3[h, :QK] * SCALE, w3[h, QK:2 * QK]], axis=0)
         for h in range(H)],
        axis=0)                     # (512, 384)
    v_rows = np.concatenate([w3[h, 2 * QK:] for h in range(H)], axis=0)  # (1024,384)
    wqkvT = np.ascontiguousarray(
        np.concatenate([qk_rows, v_rows], axis=0).T).astype(BF16_NP)  # (384,1536)

    wprojT = np.ascontiguousarray(Wproj.T).astype(BF16_NP)  # (1024, 384)
    mt = np.ascontiguousarray(interp_matrix(L, N).T).astype(BF16_NP)  # (625, 784)
    ab_all = np.ascontiguousarray(ab_table[:, bias_idxs]).astype(BF16_NP)
    bproj2 = np.ascontiguousarray(bproj.reshape(1, DIM))

    in_maps = []
    for c in range(N_CORES):
        xT = np.ascontiguousarray(
            x[c * BC:(c + 1) * BC].transpose(0, 2, 1)).astype(BF16_NP)
        in_maps.append({
            "xT": xT,
            "wqkvT": wqkvT,
            "wprojT": wprojT,
            "bproj": bproj2,
            "mt": mt,
            "ab": ab_all[c],
        })
    return in_maps


def _run(inputs, trace=False, **kw):
    nc = build()
    in_maps = _prep_in_maps(inputs)
    res = run_bass_kernel_spmd(nc, in_maps, core_ids=list(range(N_CORES)),
                               trace=trace, **kw)
    out = np.concatenate([res.results[c]["out"] for c in range(N_CORES)], axis=0)
    return out, res


def kernel(**inputs) -> np.ndarray:
    out, _ = _run(inputs, trace=False)
    return out


# revision 33
# speedup vs baseline: 1.0630x; 1.0561x over previous
"""Trainium2 Bass kernel for nn_Attention_31705448579931.

Multi-head attention (b=16, L=784, dim=384, H=8, qk=32, v=128) with a
bicubic-resampled relative-position bias:

    out = proj( softmax(q k^T/sqrt(d) + M ab M^T) v )

Sharding: data-parallel over batch — each of the 8 NeuronCores handles 2
batches and all 8 heads. The (batch-independent) bias interpolation is
replicated per core, computed head-at-a-time and fused into softmax via
exp(S+B) = exp(S) * exp(B).

Device layout highlights:
  - attention is computed k-major: S^T tiles (kpos on partitions, q on free)
    so exp(S^T) is directly the lhsT-side operand for the P@V matmul.
  - softmax denominators via an ones-vector matmul on the TensorEngine,
    reciprocal on VectorE, partition-broadcast on GpSimd, and a column-scale
    fused into the PSUM->SBUF eviction of the attention output.
  - the bias interp phase 2 exploits the 4-tap bicubic structure of the
    interpolation matrix (contraction 625 -> ~96 rows per output tile).

All matmuls run in bf16 (inputs pre-cast on host) except the two bias-interp
phases which run float32r over f32 data.
"""

import numpy as np
import ml_dtypes

import concourse.bass as bass
import concourse.mybir as mybir
import concourse.tile as tile
from concourse import bacc
from concourse.bass_utils import run_bass_kernel_spmd

N_CORES = 8
B = 16          # global batch
BC = B // N_CORES  # batches per core
L = 784
DIM = 384
H = 8
QK = 32
VD = 128
RES = 25
N = RES * RES   # 625
A_CUBIC = -0.75
SCALE = QK ** -0.5

LT = 7          # l tiles of 112
LTS = 112
NT = 5          # 625 tiles of 125
NTS = 125
F32 = mybir.dt.float32
F32R = mybir.dt.float32r
BF16 = mybir.dt.bfloat16
BF16_NP = ml_dtypes.bfloat16

NSPLITS = [(0, 512), (512, L)]  # free-dim chunks for 784-wide matmul outputs


def _cubic_weight(x):
    ax = np.abs(x)
    a = A_CUBIC
    w1 = ((a + 2.0) * ax - (a + 3.0)) * ax * ax + 1.0
    w2 = a * (((ax - 5.0) * ax + 8.0) * ax - 4.0)
    return np.where(ax <= 1.0, w1, np.where(ax < 2.0, w2, 0.0)).astype(np.float32)


def interp_matrix(Lo, Li):
    """Dense 1-D bicubic resampling matrix (Lo, Li), matches reference."""
    scale = Li / Lo
    src = (np.arange(Lo, dtype=np.float32) + 0.5) * scale - 0.5
    f = np.floor(src)
    t = (src - f).astype(np.float32)
    ws = np.stack(
        [_cubic_weight(t + 1.0), _cubic_weight(t), _cubic_weight(1.0 - t),
         _cubic_weight(2.0 - t)], axis=1)
    idx = f.astype(np.int32)[:, None] + np.arange(-1, 3, dtype=np.int32)[None, :]
    idx = np.clip(idx, 0, Li - 1)
    M = np.zeros((Lo, Li), dtype=np.float32)
    np.add.at(M, (np.arange(Lo)[:, None], idx), ws)
    return M


def _tap_windows():
    """Per l-tile row window [lo, hi) of MT rows feeding that tile (4-tap)."""
    scale = N / L
    src = (np.arange(L, dtype=np.float64) + 0.5) * scale - 0.5
    f = np.floor(src).astype(np.int64)
    lo_tap = np.clip(f - 1, 0, N - 1)
    hi_tap = np.clip(f + 2, 0, N - 1)
    wins = []
    for pt in range(LT):
        sl = slice(pt * LTS, (pt + 1) * LTS)
        wins.append((int(lo_tap[sl].min()), int(hi_tap[sl].max()) + 1))
    return wins


_BUILD_CACHE = {}


def build():
    if "nc" in _BUILD_CACHE:
        return _BUILD_CACHE["nc"]

    nc = bacc.Bacc("TRN2", target_bir_lowering=False, debug=False,
                   num_devices=N_CORES)

    xT_e = nc.dram_tensor("xT", [BC, DIM, L], BF16, kind="ExternalInput")
    wqkvT_e = nc.dram_tensor("wqkvT", [DIM, 1536], BF16, kind="ExternalInput")
    wprojT_e = nc.dram_tensor("wprojT", [H * VD, DIM], BF16, kind="ExternalInput")
    bproj_e = nc.dram_tensor("bproj", [1, DIM], F32, kind="ExternalInput")
    mt_e = nc.dram_tensor("mt", [N, L], BF16, kind="ExternalInput")
    ab_e = nc.dram_tensor("ab", [N, N], BF16, kind="ExternalInput")
    ebloc_e = nc.dram_tensor("ebloc", [L, L], BF16)
    eball_e = nc.dram_tensor("eball", [H * L, L], BF16, addr_space="Shared")
    out_e = nc.dram_tensor("out", [BC, L, DIM], F32, kind="ExternalOutput")

    wins = _tap_windows()

    with tile.TileContext(nc) as tc:
        with (
            tc.tile_pool(name="const", bufs=1) as constp,
            tc.tile_pool(name="wq", bufs=1) as wqp,
            tc.tile_pool(name="x", bufs=1) as xp,
            tc.tile_pool(name="v", bufs=1) as vp,
            tc.tile_pool(name="qk", bufs=1) as qkp,
            tc.tile_pool(name="bias", bufs=1) as biasp,
            tc.tile_pool(name="attn", bufs=2) as attnp,
            tc.tile_pool(name="flow", bufs=8) as flowp,
            tc.tile_pool(name="mis", bufs=1) as misp,
            tc.tile_pool(name="ot", bufs=1) as otp,
            tc.tile_pool(name="ps", bufs=4, space="PSUM") as psp,
        ):
            # ---- constants -------------------------------------------------
            bproj_row = constp.tile([1, DIM], F32, tag="bprow")
            nc.sync.dma_start(bproj_row[:], bproj_e[:, :])
            bpb = constp.tile([128, DIM], F32, tag="bpb")
            nc.gpsimd.partition_broadcast(bpb[:], bproj_row[:])

            ones_l = constp.tile([LTS, VD], BF16, tag="ones")
            nc.any.memset(ones_l[:], 1.0)

            # weights
            wq_sb = []
            for kc in range(3):
                t = wqp.tile([128, 1536], BF16, tag=f"wq{kc}")
                nc.sync.dma_start(t[:], wqkvT_e[kc * 128:(kc + 1) * 128, :])
                wq_sb.append(t)
            wproj_sb = []
            for h in range(H):
                t = wqp.tile([128, DIM], BF16, tag=f"wp{h}")
                nc.sync.dma_start(t[:], wprojT_e[h * VD:(h + 1) * VD, :])
                wproj_sb.append(t)


            # x^T
            xT_sb = [[None] * 3 for _ in range(BC)]
            for b in range(BC):
                for kc in range(3):
                    t = xp.tile([128, L], BF16, tag=f"x{b}{kc}")
                    nc.sync.dma_start(t[:], xT_e[b, kc * 128:(kc + 1) * 128, :])
                    xT_sb[b][kc] = t

            # ---- PE warmup: ~4us of dense matmuls releases the HAM clock
            # throttle (1.2 -> 2.4 GHz) before the latency-critical bias
            # phase that gates the AllGather.
            warm_ps = psp.tile([VD, 512], F32, tag="ps")
            for wi in range(20):
                nc.tensor.matmul(
                    warm_ps[:],
                    lhsT=ones_l[:],
                    rhs=wq_sb[0][:LTS, 0:512],
                    start=(wi == 0), stop=(wi == 19),
                )

            # ---- own-head bias: expB^T = exp(M ab^T M^T), then AllGather --
            # scoped pool: all bias-phase operands are freed once the
            # collective is issued.
            with tc.tile_pool(name="bias1", bufs=1) as b1p, \
                    tc.tile_pool(name="uwp", bufs=2) as uwp:
                mt_sb = []
                for ic in range(NT):
                    t = b1p.tile([NTS, L], BF16, tag=f"mt{ic}")
                    nc.scalar.dma_start(t[:], mt_e[ic * NTS:(ic + 1) * NTS, :])
                    mt_sb.append(t)
                mtwin_sb = []
                for pt in range(LT):
                    lo, hi = wins[pt]
                    t = b1p.tile([hi - lo, LTS], BF16, tag=f"mtw{pt}")
                    nc.scalar.dma_start(t[:], mt_e[lo:hi, pt * LTS:(pt + 1) * LTS])
                    mtwin_sb.append(t)
                ab_sb = []
                for ic in range(NT):
                    t = b1p.tile([NTS, N], BF16, tag=f"ab{ic}")
                    nc.gpsimd.dma_start(t[:], ab_e[ic * NTS:(ic + 1) * NTS, :])
                    ab_sb.append(t)
                for pt in range(LT):
                    lo, hi = wins[pt]
                    pu = psp.tile([hi - lo, L], F32, tag="ps")
                    for (ns, ne) in NSPLITS:
                        for ic in range(NT):
                            nc.tensor.matmul(
                                pu[:, ns:ne],
                                lhsT=ab_sb[ic][:, lo:hi],
                                rhs=mt_sb[ic][:, ns:ne],
                                start=(ic == 0), stop=(ic == NT - 1),
                            )
                    uw = uwp.tile([hi - lo, L], BF16, tag="uw")
                    nc.vector.tensor_copy(uw[:], pu[:])
                    pb = psp.tile([LTS, L], F32, tag="ps")
                    for (ns, ne) in NSPLITS:
                        nc.tensor.matmul(
                            pb[:, ns:ne],
                            lhsT=mtwin_sb[pt][:],
                            rhs=uw[:, ns:ne],
                            start=True, stop=True,
                        )
                    ebo = b1p.tile([LTS, L], BF16, tag="ebo")
                    nc.scalar.activation(ebo[:], pb[:],
                                         mybir.ActivationFunctionType.Exp)
                    nc.gpsimd.dma_start(ebloc_e[pt * LTS:(pt + 1) * LTS, :], ebo[:])
                nc.gpsimd.collective_compute(
                    "AllGather",
                    mybir.AluOpType.bypass,
                    replica_groups=[list(range(N_CORES))],
                    ins=[ebloc_e.ap().opt()],
                    outs=[eball_e.ap().opt()],
                )

            # ---- V projection (l-major: kpos on partitions) ---------------
            v_sb = [[None] * LT for _ in range(BC)]
            for b in range(BC):
                for lt in range(LT):
                    pv = psp.tile([LTS, 1024], F32, tag="ps")
                    for half in range(2):
                        for kc in range(3):
                            nc.tensor.matmul(
                                pv[:, half * 512:(half + 1) * 512],
                                lhsT=xT_sb[b][kc][:, lt * LTS:(lt + 1) * LTS],
                                rhs=wq_sb[kc][:, 512 + half * 512:1024 + half * 512],
                                start=(kc == 0), stop=(kc == 2),
                            )
                    vt = vp.tile([LTS, 1024], BF16, tag=f"v{b}{lt}")
                    nc.scalar.copy(vt[:], pv[:])
                    v_sb[b][lt] = vt

            # ---- q/k projections for all heads (hoisted so this PE work
            # overlaps the AllGather) ---------------------------------------
            qk_all = {}
            for mt_i in range(H // 2):
                for b in range(BC):
                    pqk = psp.tile([128, L], F32, tag="ps")
                    for (ns, ne) in NSPLITS:
                        for kc in range(3):
                            nc.tensor.matmul(
                                pqk[:, ns:ne],
                                lhsT=wq_sb[kc][:, mt_i * 128:(mt_i + 1) * 128],
                                rhs=xT_sb[b][kc][:, ns:ne],
                                start=(kc == 0), stop=(kc == 2),
                            )
                    for hh in range(2):
                        h_ = 2 * mt_i + hh
                        qt = qkp.tile([2 * QK, L], BF16, tag=f"q{b}{h_}")
                        kt_ = qkp.tile([2 * QK, L], BF16, tag=f"k{b}{h_}")
                        nc.vector.tensor_copy(qt[0:QK, :], pqk[hh * 64:hh * 64 + 32, :])
                        nc.vector.tensor_copy(kt_[0:QK, :], pqk[hh * 64 + 32:hh * 64 + 64, :])
                        # duplicate rows into partitions 32:64 for 2-way
                        # row-group packing of the S^T matmuls
                        nc.sync.dma_start(qt[QK:2 * QK, :], qt[0:QK, :])
                        nc.sync.dma_start(kt_[QK:2 * QK, :], kt_[0:QK, :])
                        qk_all[(b, h_)] = (qt, kt_)

            # ---- per-head loop --------------------------------------------
            ot_sb = [[None] * H for _ in range(BC)]
            _pend = []

            def _flush_pv(item):
                fh, fb, ptiles = item
                ps_o = psp.tile([VD, L], F32, tag="ps")
                ps_one = psp.tile([VD, L], F32, tag="ps")
                # ones chains first (M=128: same N-bound cost as M=1, but the
                # output replicates the column sums across all partitions --
                # no partition broadcast needed and the reciprocal runs on
                # all 128 lanes).
                for (ns, ne) in NSPLITS:
                    for kt in range(LT):
                        nc.tensor.matmul(
                            ps_one[:, ns:ne],
                            lhsT=ones_l[:],
                            rhs=ptiles[kt][:, ns:ne],
                            start=(kt == 0), stop=(kt == LT - 1),
                        )
                rdb = misp.tile([VD, L], F32, tag="rdb")
                nc.vector.reciprocal_approx_fast(rdb[:], ps_one[:])
                for (ns, ne) in NSPLITS:
                    for kt in range(LT):
                        nc.tensor.matmul(
                            ps_o[:, ns:ne],
                            lhsT=v_sb[fb][kt][:, fh * VD:(fh + 1) * VD],
                            rhs=ptiles[kt][:, ns:ne],
                            start=(kt == 0), stop=(kt == LT - 1),
                        )
                ot = otp.tile([VD, L], BF16, tag=f"ot{fb}{fh}")
                nc.vector.tensor_mul(ot[:], ps_o[:], rdb[:])
                ot_sb[fb][fh] = ot
            for h in range(H):
                # --- bias for head h: load gathered expB^T tiles -----------
                expb_sb = []
                for pt in range(LT):
                    eb = biasp.tile([LTS, L], BF16, tag=f"eb{pt}")
                    nc.gpsimd.dma_start(
                        eb[:], eball_e[h * L + pt * LTS:h * L + (pt + 1) * LTS, :])
                    expb_sb.append(eb)

                # --- attention for (h, b): software-pipelined ---------------
                # Emit instance i's S^T/exp/mul, then flush instance i-1's
                # PV/ones chains: by the time the PE reaches a chain, every
                # pT operand is long ready, so the chain streams gap-free.
                for b in range(BC):
                    qt, kt_ = qk_all[(b, h)]
                    pt_tiles = []
                    for w0 in range(0, LT, 2):
                        wave = [w0] if w0 + 1 >= LT else [w0, w0 + 1]
                        pss = []
                        for wi, kt in enumerate(wave):
                            ps_s = psp.tile([LTS, L], F32, tag="ps")
                            pss.append(ps_s)
                        # row-group packed: wave member wi uses partitions
                        # [32*wi, 32*wi+32) of the duplicated q/k tiles, so
                        # the two K=32 matmuls run concurrently on the PE.
                        for (ns, ne) in NSPLITS:
                            for wi, kt in enumerate(wave):
                                o0 = wi * QK
                                nc.tensor.matmul(
                                    pss[wi][:, ns:ne],
                                    lhsT=kt_[o0:o0 + QK, kt * LTS:(kt + 1) * LTS],
                                    rhs=qt[o0:o0 + QK, ns:ne],
                                    start=True, stop=True,
                                )
                        for wi, kt in enumerate(wave):
                            es = flowp.tile([LTS, L], BF16, tag="expS")
                            nc.scalar.activation(es[:], pss[wi][:],
                                                 mybir.ActivationFunctionType.Exp)
                            ptile = attnp.tile([LTS, L], BF16, tag=f"pT{kt}")
                            nc.vector.tensor_mul(ptile[:], es[:], expb_sb[kt][:])
                            pt_tiles.append(ptile)
                    _pend.append((h, b, pt_tiles))
                    if len(_pend) > 1:
                        _flush_pv(_pend.pop(0))

            while _pend:
                _flush_pv(_pend.pop(0))

            # ---- output projection ----------------------------------------
            for b in range(BC):
                for lt in range(LT):
                    py = psp.tile([LTS, DIM], F32, tag="ps")
                    for h in range(H):
                        nc.tensor.matmul(
                            py[:],
                            lhsT=ot_sb[b][h][:, lt * LTS:(lt + 1) * LTS],
                            rhs=wproj_sb[h][:],
                            start=(h == 0), stop=(h == H - 1),
                        )
                    ysb = misp.tile([LTS, DIM], F32, tag="y")
                    nc.vector.tensor_add(ysb[:], py[:], bpb[:LTS, :])
                    nc.sync.dma_start(
                        out_e[b, lt * LTS:(lt + 1) * LTS, :], ysb[:])

    nc.compile()
    _BUILD_CACHE["nc"] = nc
    return nc


def _prep_in_maps(inputs):
    x = np.asarray(inputs["x"], dtype=np.float32)
    Wqkv = np.asarray(inputs["Wqkv"], dtype=np.float32)
    Wproj = np.asarray(inputs["Wproj"], dtype=np.float32)
    bproj = np.asarray(inputs["bproj"], dtype=np.float32)
    ab_table = np.asarray(inputs["ab_table"], dtype=np.float32)
    bias_idxs = np.asarray(inputs["bias_idxs"])

    # reorder qkv weight rows: [q0 k0 q1 k1 ... q7 k7 | v0 v1 ... v7]
    w3 = Wqkv.reshape(H, 2 * QK + VD, DIM)
    order = []
    for h in range(H):
        pass
    qk_rows = np.concatenate(
        [np.concatenate([w3[h, :QK] * SCALE, w3[h, QK:2 * QK]], axis=0)
         for h in range(H)],
        axis=0)                     # (512, 384)
    v_rows = np.concatenate([w3[h, 2 * QK:] for h in range(H)], axis=0)  # (1024,384)
    wqkvT = np.ascontiguousarray(
        np.concatenate([qk_rows, v_rows], axis=0).T).astype(BF16_NP)  # (384,1536)

    wprojT = np.ascontiguousarray(Wproj.T).astype(BF16_NP)  # (1024, 384)
    mt = np.ascontiguousarray(interp_matrix(L, N).T).astype(BF16_NP)  # (625, 784)
    ab_all = np.ascontiguousarray(ab_table[:, bias_idxs]).astype(BF16_NP)
    bproj2 = np.ascontiguousarray(bproj.reshape(1, DIM))

    in_maps = []
    for c in range(N_CORES):
        xT = np.ascontiguousarray(
            x[c * BC:(c + 1) * BC].transpose(0, 2, 1)).astype(BF16_NP)
        in_maps.append({
            "xT": xT,
            "wqkvT": wqkvT,
            "wprojT": wprojT,
            "bproj": bproj2,
            "mt": mt,
            "ab": ab_all[c],
        })
    return in_maps


def _run(inputs, trace=False, **kw):
    nc = build()
    in_maps = _prep_in_maps(inputs)
    res = run_bass_kernel_spmd(nc, in_maps, core_ids=list(range(N_CORES)),
                               trace=trace, **kw)
    out = np.concatenate([res.results[c]["out"] for c in range(N_CORES)], axis=0)
    return out, res


def kernel(**inputs) -> np.ndarray:
    out, _ = _run(inputs, trace=False)
    return out
